# revision 3
# baseline (speedup 1.0000x reference)
"""CartBasisStressHead kernel for Trainium2 (8 NeuronCores, SPMD data-parallel).

Strategy
--------
Only 6 of the 9 m-rows of node_embedding are used: row 0 feeds a SiLU MLP
(per-node scalar), rows 4:9 feed a per-channel contraction (l=2 branch).
Nodes are sharded contiguously across 8 cores (graphs may straddle shard
boundaries; segment sums are linear, so host adds the partials).

The l=2 data is pre-scaled by w_l2 on the host (folded into the bf16 cast),
so the device only has to segment-sum it: a 0/1 indicator matrix
A[node, local_graph] (built on the vector engine from iota == local graph
id) is the stationary matmul operand, so S[g_local, (m,c)] accumulates in
PSUM across each 1024-node group's 8 node-tiles; two DVE tensor_reduce ops
then collapse c, leaving a tiny (W, 5) per-group partial that is staged in
SBUF and stored once at the end (24 KB instead of megabytes of S partials).

The scalar branch: h = silu(W1 @ x0T + b1); h = silu(W2 @ h + b2);
s = W3 @ h (feature-major, x0 transposed on host). Per-node scalars are
copied off PSUM and stored per 4-group batch.

Matmul data travels in bf16 (fp32 accumulation in PSUM). Inputs stream in
2-group superchunks (~3.1 MB el=2, 0.5 MB x0) so nearly all DMA time is
large transfers at the ~360 GB/s per-core HBM roofline.

Host epilogue: bincount segment-sum of per-node scalars, scatter-add of the
tiny per-group aniso partials, and the (G,9)@(9,9) change-of-basis.
"""

import sys

if "/opt/trn_rl_repo" not in sys.path:
    sys.path.insert(0, "/opt/trn_rl_repo")

import numpy as np
import ml_dtypes

import concourse.bacc as bacc
import concourse.tile as tile
from concourse import mybir
from concourse import bass_utils

_S2 = 2.0 ** -0.5
_S3 = 3.0 ** -0.5
_S6 = 6.0 ** -0.5
_CG = np.array([
    [_S3, 0, 0, 0, _S3, 0, 0, 0, _S3],
    [0, 0, 0, 0, 0, _S2, 0, -_S2, 0],
    [0, 0, -_S2, 0, 0, 0, _S2, 0, 0],
    [0, _S2, 0, -_S2, 0, 0, 0, 0, 0],
    [0, 0, _S2, 0, 0, 0, _S2, 0, 0],
    [0, 0, 0, 0, 0, _S2, 0, _S2, 0],
    [-_S6, 0, 0, 0, 2 * _S6, 0, 0, 0, -_S6],
    [0, _S2, 0, _S2, 0, 0, 0, 0, 0],
    [-_S2, 0, 0, 0, 0, 0, 0, 0, _S2],
], dtype=np.float32)  # (9, 9)

N_CORES = 8
P = 128          # SBUF partitions
NG = 1024        # nodes per group (one PSUM accumulation span)
T = NG // P      # node-tiles per group
ML2 = 5 * P      # 640 values of l=2 data per node
SC = 2           # groups per input superchunk
OB = 4           # groups per scalar-output staging batch

F32 = mybir.dt.float32
BF16 = mybir.dt.bfloat16
WIRE = ml_dtypes.bfloat16

_BUILD_CACHE = {}


def _build(n_pad, n_groups, W, n_real):
    key = (n_pad, n_groups, W, n_real)
    if key in _BUILD_CACHE:
        return _BUILD_CACHE[key]

    n_sc = (n_groups + SC - 1) // SC
    n_ob = (n_groups + OB - 1) // OB

    nc = bacc.Bacc("TRN2", target_bir_lowering=False, debug=False,
                   num_devices=N_CORES)

    x0T = nc.dram_tensor("x0T", (P, n_pad), BF16,
                         kind="ExternalInput").ap()
    # host pre-tiled and pre-scaled by w_l2: [sc, p, 2*T*640] contiguous
    embL2 = nc.dram_tensor("embL2", (n_sc, P, SC * T * ML2), BF16,
                           kind="ExternalInput").ap()
    lgid = nc.dram_tensor("lgid", (P, n_groups * T), BF16,
                          kind="ExternalInput").ap()
    iota_in = nc.dram_tensor("iota_in", (P, W), BF16, kind="ExternalInput").ap()
    w1t = nc.dram_tensor("w1t", (P, P), BF16, kind="ExternalInput").ap()
    w2t = nc.dram_tensor("w2t", (P, P), BF16, kind="ExternalInput").ap()
    w3t = nc.dram_tensor("w3t", (P, 1), BF16, kind="ExternalInput").ap()
    b1 = nc.dram_tensor("b1c", (P, 1), F32, kind="ExternalInput").ap()
    b2 = nc.dram_tensor("b2c", (P, 1), F32, kind="ExternalInput").ap()
    scal = nc.dram_tensor("scal", (n_ob, OB * NG), F32,
                          kind="ExternalOutput").ap()
    aniso_o = nc.dram_tensor("aniso_o", (48, n_groups * 5), F32,
                             kind="ExternalOutput").ap()

    silu = mybir.ActivationFunctionType.Silu
    eq = mybir.AluOpType.is_equal

    with tile.TileContext(nc) as tc:
        with (
            tc.tile_pool(name="const", bufs=1) as cpool,
            tc.tile_pool(name="x0p", bufs=3) as x0p,
            tc.tile_pool(name="el2p", bufs=3) as el2p,
            tc.tile_pool(name="hp", bufs=3) as hp,
            tc.tile_pool(name="stp", bufs=2) as stp,
            tc.tile_pool(name="ph1", bufs=2, space="PSUM") as ph1p,
            tc.tile_pool(name="ph2", bufs=1, space="PSUM") as ph2p,
            tc.tile_pool(name="psc", bufs=2, space="PSUM") as pscp,
            tc.tile_pool(name="pS", bufs=2, space="PSUM") as pSp,
        ):
            w1s = cpool.tile([P, P], BF16)
            w2s = cpool.tile([P, P], BF16)
            w3s = cpool.tile([P, 1], BF16)
            b1s = cpool.tile([P, 1], F32)
            b2s = cpool.tile([P, 1], F32)
            iotas = cpool.tile([P, W], BF16)
            lgids = cpool.tile([P, n_groups * T], BF16)
            anisoSt = cpool.tile([48, n_groups * 5], F32)
            nc.scalar.dma_start(out=w1s[:], in_=w1t)
            nc.scalar.dma_start(out=w2s[:], in_=w2t)
            nc.scalar.dma_start(out=w3s[:], in_=w3t)
            nc.scalar.dma_start(out=b1s[:], in_=b1)
            nc.scalar.dma_start(out=b2s[:], in_=b2)
            nc.scalar.dma_start(out=iotas[:], in_=iota_in)
            nc.scalar.dma_start(out=lgids[:], in_=lgid)

            # all indicator matrices up front (depend only on constants),
            # so aniso matmuls never wait on the vector engine
            Aall = cpool.tile([P, n_groups * T * W], BF16)
            for grp in range(n_groups):
                nc.vector.tensor_tensor(
                    out=Aall[:, grp * T * W: (grp + 1) * T * W]
                        .rearrange("p (t w) -> p t w", t=T, w=W),
                    in0=iotas[:].unsqueeze(1).to_broadcast([P, T, W]),
                    in1=lgids[:, grp * T: (grp + 1) * T]
                        .unsqueeze(2).to_broadcast([P, T, W]),
                    op=eq)

            x0c = el2c = None
            scst = None
            for grp in range(n_groups):
                grp_real = min(NG, n_real - grp * NG)
                Tr = (grp_real + P - 1) // P
                Sr = (grp_real + 511) // 512

                if grp % SC == 0:
                    sc = grp // SC
                    sc_real = min(SC * NG, n_real - sc * SC * NG)
                    Src = (sc_real + 511) // 512
                    Trc = (sc_real + P - 1) // P
                    x0c = x0p.tile([P, SC * NG], BF16, tag="x0c")
                    nc.sync.dma_start(
                        out=x0c[:, :Src * 512],
                        in_=x0T[:, sc * SC * NG: sc * SC * NG + Src * 512])
                    el2c = el2p.tile([P, SC * T * ML2], BF16, tag="el2c")
                    nc.sync.dma_start(
                        out=el2c[:, :Trc * ML2],
                        in_=embL2[sc][:, :Trc * ML2])
                j = grp % SC
                goff = j * NG
                toff = j * T

                if grp % OB == 0:
                    scst = stp.tile([1, OB * NG], F32, tag="scst")
                boff = grp % OB

                # ---- scalar (MLP) branch, 512 nodes at a time ----
                h2list = []
                for s in range(Sr):
                    nsl = slice(goff + s * 512, goff + (s + 1) * 512)
                    h1p = ph1p.tile([P, 512], F32, tag="h1p")
                    nc.tensor.matmul(h1p[:], w1s[:], x0c[:, nsl],
                                     start=True, stop=True)
                    h1s = hp.tile([P, 512], BF16, tag="h1s")
                    nc.scalar.activation(h1s[:], h1p[:], silu, bias=b1s[:])
                    h2p = ph2p.tile([P, 512], F32, tag="h2p")
                    nc.tensor.matmul(h2p[:], w2s[:], h1s[:],
                                     start=True, stop=True)
                    h2s = hp.tile([P, 512], BF16, tag="h2s")
                    nc.scalar.activation(h2s[:], h2p[:], silu, bias=b2s[:])
                    h2list.append(h2s)

                # ---- l=2 branch with fused segment sum ----
                # cols 0:256 = (m0,m1) -> pS[0:W]; 256:640 = (m2..m4)
                # -> pS[32:32+W]; both accumulate over the group's tiles.
                A8 = Aall[:, grp * T * W: (grp + 1) * T * W]
                pS = pSp.tile([64, 384], F32, tag="pS")
                for t in range(Tr):
                    At = A8[:, t * W:(t + 1) * W]
                    base = (toff + t) * ML2
                    nc.tensor.matmul(pS[0:W, 0:256], At,
                                     el2c[:, base: base + 256],
                                     start=(t == 0), stop=(t == Tr - 1),
                                     tile_position=(0, 0))
                    nc.tensor.matmul(pS[32:32 + W, 0:384], At,
                                     el2c[:, base + 256: base + ML2],
                                     start=(t == 0), stop=(t == Tr - 1),
                                     tile_position=(0, 32))
                    if t == 0:
                        scp = pscp.tile([P, 512], F32, tag="scp")
                        for s in range(Sr):
                            q = 64 + 32 * s
                            nc.tensor.matmul(scp[q:q + 1, :], w3s[:],
                                             h2list[s][:],
                                             start=True, stop=True,
                                             tile_position=(0, q))

                # collapse c on the vector engine -> (W, 5) partial
                nc.vector.tensor_reduce(
                    out=anisoSt[0:W, grp * 5: grp * 5 + 2],
                    in_=pS[0:W, 0:256].rearrange("p (f c) -> p f c",
                                                 f=2, c=P),
                    axis=mybir.AxisListType.X,
                    op=mybir.AluOpType.add)
                nc.vector.tensor_reduce(
                    out=anisoSt[32:32 + W, grp * 5 + 2: grp * 5 + 5],
                    in_=pS[32:32 + W, 0:384].rearrange("p (f c) -> p f c",
                                                       f=3, c=P),
                    axis=mybir.AxisListType.X,
                    op=mybir.AluOpType.add)

                for s in range(Sr):
                    q = 64 + 32 * s
                    nc.vector.tensor_copy(
                        out=scst[:, boff * NG + s * 512:
                                 boff * NG + (s + 1) * 512],
                        in_=scp[q:q + 1, :])

                if grp % OB == OB - 1 or grp == n_groups - 1:
                    ob = grp // OB
                    nc.sync.dma_start(out=scal[ob: ob + 1, :], in_=scst[:])

            nc.sync.dma_start(out=aniso_o, in_=anisoSt[:])

    nc.compile()
    _BUILD_CACHE[key] = nc
    return nc


def _next_pow2(x):
    p = 8
    while p < x:
        p *= 2
    return p


def _host_reference(node_embedding, W1, b1, W2, b2, W3, b3, w_l2, batch,
                    natoms):
    """Pure-numpy fallback (only used for pathological graph layouts)."""
    G = natoms.shape[0]
    inv = 1.0 / natoms.astype(np.float32)
    x = node_embedding[:, 0, :]
    h = x @ W1.T + b1
    h = h / (1.0 + np.exp(-h))
    h = h @ W2.T + b2
    h = h / (1.0 + np.exp(-h))
    ns = (h @ W3.T + b3)[:, 0]
    ok = (batch >= 0) & (batch < G)
    bok = batch[ok]
    iso = np.bincount(bok, weights=ns[ok], minlength=G).astype(np.float32) \
        * inv
    nl2 = np.einsum("nmc,c->nm", node_embedding[:, 4:9, :], w_l2[0])
    aniso = np.stack(
        [np.bincount(bok, weights=nl2[ok, m], minlength=G)
         for m in range(5)], axis=1).astype(np.float32) * inv[:, None]
    dec = np.concatenate([iso[:, None], np.zeros((G, 3), np.float32), aniso],
                         axis=1)
    return (dec @ _CG).reshape(-1, 3, 3).astype(np.float32)


def kernel(node_embedding, W1, b1, W2, b2, W3, b3, w_l2, batch, natoms):
    node_embedding = np.asarray(node_embedding, dtype=np.float32)
    W1 = np.asarray(W1, dtype=np.float32)
    b1 = np.asarray(b1, dtype=np.float32)
    W2 = np.asarray(W2, dtype=np.float32)
    b2 = np.asarray(b2, dtype=np.float32)
    W3 = np.asarray(W3, dtype=np.float32)
    b3 = np.asarray(b3, dtype=np.float32)
    w_l2 = np.asarray(w_l2, dtype=np.float32)
    batch = np.asarray(batch).astype(np.int64)
    natoms_in = np.asarray(natoms)

    N = node_embedding.shape[0]
    G = natoms_in.shape[0]
    n_sh = (N + N_CORES - 1) // N_CORES
    n_groups = (n_sh + NG - 1) // NG
    n_pad = n_groups * NG
    n_sc = (n_groups + SC - 1) // SC

    # per-core shard ranges and group graph bases
    shards = []
    W_need = 8
    for c in range(N_CORES):
        n0 = min(c * n_sh, N)
        n1 = min(n0 + n_sh, N)
        b = batch[n0:n1]
        nreal = n1 - n0
        gbase = np.zeros(n_groups, np.int64)
        for grp in range(n_groups):
            lo = grp * NG
            hi = min(lo + NG, nreal)
            if lo < nreal:
                gbase[grp] = b[lo]
                span = int(b[hi - 1] - b[lo] + 1)
                W_need = max(W_need, span)
        shards.append((n0, n1, b, gbase))
    W = _next_pow2(W_need)
    if (W > 32 or not np.all(batch[:-1] <= batch[1:])
            or batch.min(initial=0) < 0 or batch.max(initial=0) >= G):
        return _host_reference(node_embedding, W1, b1, W2, b2, W3, b3,
                               w_l2, batch, natoms_in)

    nc = _build(n_pad, n_groups, W, n_sh)

    w1t = np.ascontiguousarray(W1.T).astype(WIRE)
    w2t = np.ascontiguousarray(W2.T).astype(WIRE)
    w3t = np.ascontiguousarray(W3.T).astype(WIRE)
    b1c = np.ascontiguousarray(b1[:, None])
    b2c = np.ascontiguousarray(b2[:, None])
    iota_c = np.ascontiguousarray(
        np.tile(np.arange(W, dtype=np.float32), (P, 1))).astype(WIRE)

    in_maps = []
    for c in range(N_CORES):
        n0, n1, b, gbase = shards[c]
        nreal = n1 - n0
        x0T = np.zeros((P, n_pad), WIRE)
        x0T[:, :nreal] = node_embedding[n0:n1, 0, :].T.astype(WIRE)
        # pre-tiled l=2 data scaled by w_l2: [sc, p, (2T, m)] so each
        # partition's superchunk read is one contiguous run
        el2 = np.zeros((n_sc * SC * NG, ML2), WIRE)
        el2[:nreal] = (node_embedding[n0:n1, 4:9, :]
                       * w_l2[0]).reshape(nreal, ML2).astype(WIRE)
        el2 = np.ascontiguousarray(
            el2.reshape(n_sc, SC * T, P, ML2).transpose(0, 2, 1, 3)
               .reshape(n_sc, P, SC * T * ML2))
        lg = np.full(n_pad, -1.0, np.float32)
        lg[:nreal] = (b - np.repeat(gbase, NG)[:nreal]).astype(np.float32)
        lg_t = np.ascontiguousarray(
            lg.reshape(n_groups, T, P).transpose(2, 0, 1)
              .reshape(P, n_groups * T)).astype(WIRE)
        in_maps.append({
            "x0T": x0T, "embL2": el2, "lgid": lg_t, "iota_in": iota_c,
            "w1t": w1t, "w2t": w2t, "w3t": w3t, "b1c": b1c, "b2c": b2c,
        })

    res = bass_utils.run_bass_kernel_spmd(nc, in_maps,
                                          core_ids=list(range(N_CORES)))

    # ---- host epilogue ----
    inv = (1.0 / natoms_in.astype(np.float32)).astype(np.float32)
    node_scalar = np.empty(N, np.float32)
    aniso = np.zeros((G + 64, 5), np.float32)
    for c in range(N_CORES):
        n0, n1, _, gbase = shards[c]
        nreal = n1 - n0
        sc = res.results[c]["scal"].reshape(-1)[:nreal]
        node_scalar[n0:n1] = sc
        an = res.results[c]["aniso_o"]       # (48, n_groups*5)
        for grp in range(n_groups):
            if grp * NG < nreal:
                gb = int(gbase[grp])
                aniso[gb:gb + W, 0:2] += an[0:W, grp * 5: grp * 5 + 2]
                aniso[gb:gb + W, 2:5] += an[32:32 + W,
                                            grp * 5 + 2: grp * 5 + 5]
    iso = np.bincount(batch, weights=node_scalar + b3[0], minlength=G)
    iso = iso.astype(np.float32) * inv
    aniso = aniso[:G] * inv[:, None]
    dec = np.concatenate([iso[:, None], np.zeros((G, 3), np.float32), aniso],
                         axis=1)
    return (dec @ _CG).reshape(-1, 3, 3).astype(np.float32)


# revision 15
# speedup vs baseline: 1.2010x; 1.2010x over previous
"""CartBasisStressHead kernel for Trainium2 (8 NeuronCores, SPMD data-parallel).

Strategy
--------
Only 6 of the 9 m-rows of node_embedding are used: row 0 feeds a SiLU MLP
(per-node scalar), rows 4:9 feed a per-channel contraction (l=2 branch).
Nodes are sharded contiguously across 8 cores (graphs may straddle shard
boundaries; segment sums are linear, so host adds the partials).

The l=2 data is pre-scaled by w_l2 on the host (folded into the bf16 cast),
so the device only has to segment-sum it: a 0/1 indicator matrix
A[node, local_graph] (built on the vector engine from iota == local graph
id) is the stationary matmul operand, so S[g_local, (m,c)] accumulates in
PSUM across each 1024-node group's 8 node-tiles; two DVE tensor_reduce ops
then collapse c, leaving a tiny (W, 5) per-group partial that is staged in
SBUF and stored once at the end (24 KB instead of megabytes of S partials).

The scalar branch: h = silu(W1 @ x0T + b1); h = silu(W2 @ h + b2);
s = W3 @ h (feature-major, x0 transposed on host). Per-node scalars are
copied off PSUM and stored per 8-group batch (the final batch copies on
the scalar engine, which is idle at the tail).

Matmul data travels in bf16 (fp32 accumulation in PSUM). Inputs stream as
per-group DMAs (1.3 MB el=2 with 10.25 KB per-partition runs, 0.26 MB x0),
8 groups deep, which measured fastest (~380 GB/s per core) of all load
shapes tried; constants ride in two merged blobs. Output DMA count is kept
minimal because every DMA instruction costs ~115 ns per engine in the
end-of-kernel semaphore-reset march.

Host epilogue: bincount segment-sum of per-node scalars, scatter-add of the
tiny per-group aniso partials, and the (G,9)@(9,9) change-of-basis.
"""

import sys

if "/opt/trn_rl_repo" not in sys.path:
    sys.path.insert(0, "/opt/trn_rl_repo")

import numpy as np
import ml_dtypes

import concourse.bacc as bacc
import concourse.tile as tile
from concourse import mybir
from concourse import bass_utils

_S2 = 2.0 ** -0.5
_S3 = 3.0 ** -0.5
_S6 = 6.0 ** -0.5
_CG = np.array([
    [_S3, 0, 0, 0, _S3, 0, 0, 0, _S3],
    [0, 0, 0, 0, 0, _S2, 0, -_S2, 0],
    [0, 0, -_S2, 0, 0, 0, _S2, 0, 0],
    [0, _S2, 0, -_S2, 0, 0, 0, 0, 0],
    [0, 0, _S2, 0, 0, 0, _S2, 0, 0],
    [0, 0, 0, 0, 0, _S2, 0, _S2, 0],
    [-_S6, 0, 0, 0, 2 * _S6, 0, 0, 0, -_S6],
    [0, _S2, 0, _S2, 0, 0, 0, 0, 0],
    [-_S2, 0, 0, 0, 0, 0, 0, 0, _S2],
], dtype=np.float32)  # (9, 9)

N_CORES = 8
P = 128          # SBUF partitions
NG = 1024        # nodes per group (one PSUM accumulation span)
T = NG // P      # node-tiles per group
ML2 = 5 * P      # 640 values of l=2 data per node
SC = 1           # groups per input superchunk
OB = 8           # groups per scalar-output staging batch

F32 = mybir.dt.float32
BF16 = mybir.dt.bfloat16
WIRE = ml_dtypes.bfloat16

_BUILD_CACHE = {}


def _build(n_pad, n_groups, W, n_real):
    key = (n_pad, n_groups, W, n_real)
    if key in _BUILD_CACHE:
        return _BUILD_CACHE[key]

    n_sc = (n_groups + SC - 1) // SC
    n_ob = (n_groups + OB - 1) // OB

    nc = bacc.Bacc("TRN2", target_bir_lowering=False, debug=False,
                   num_devices=N_CORES)

    x0T = nc.dram_tensor("x0T", (P, n_pad), BF16,
                         kind="ExternalInput").ap()
    # host pre-tiled and pre-scaled by w_l2: [grp, p, (t, m)] so each
    # partition's group read is one contiguous 10.25 KB run
    embL2 = nc.dram_tensor("embL2", (n_groups, P, T * ML2), BF16,
                           kind="ExternalInput").ap()
    nbf = 2 * P + 1 + W + n_groups * T
    cbf_in = nc.dram_tensor("cbf", (P, nbf), BF16,
                            kind="ExternalInput").ap()
    cf32_in = nc.dram_tensor("cf32", (P, 2), F32,
                             kind="ExternalInput").ap()
    scal = nc.dram_tensor("scal", (n_ob, OB * NG), F32,
                          kind="ExternalOutput").ap()
    aniso_o = nc.dram_tensor("aniso_o", (48, n_groups * 5), F32,
                             kind="ExternalOutput").ap()

    silu = mybir.ActivationFunctionType.Silu
    eq = mybir.AluOpType.is_equal

    with tile.TileContext(nc) as tc:
        with (
            tc.tile_pool(name="const", bufs=1) as cpool,
            tc.tile_pool(name="x0p", bufs=8) as x0p,
            tc.tile_pool(name="el2p", bufs=8) as el2p,
            tc.tile_pool(name="hp", bufs=3) as hp,
            tc.tile_pool(name="stp", bufs=2) as stp,
            tc.tile_pool(name="ph1", bufs=2, space="PSUM") as ph1p,
            tc.tile_pool(name="ph2", bufs=1, space="PSUM") as ph2p,
            tc.tile_pool(name="psc", bufs=2, space="PSUM") as pscp,
            tc.tile_pool(name="pS", bufs=2, space="PSUM") as pSp,
        ):
            cbf = cpool.tile([P, nbf], BF16)
            cf32 = cpool.tile([P, 2], F32)
            anisoSt = cpool.tile([48, n_groups * 5], F32)
            nc.scalar.dma_start(out=cbf[:], in_=cbf_in)
            nc.scalar.dma_start(out=cf32[:], in_=cf32_in)
            w1s = cbf[:, 0:P]
            w2s = cbf[:, P:2 * P]
            w3s = cbf[:, 2 * P:2 * P + 1]
            iotas = cbf[:, 2 * P + 1:2 * P + 1 + W]
            lgids = cbf[:, 2 * P + 1 + W:nbf]
            b1s = cf32[:, 0:1]
            b2s = cf32[:, 1:2]

            # all indicator matrices up front (depend only on constants),
            # so aniso matmuls never wait on the vector engine
            Aall = cpool.tile([P, n_groups * T * W], BF16)
            for grp in range(n_groups):
                nc.vector.tensor_tensor(
                    out=Aall[:, grp * T * W: (grp + 1) * T * W]
                        .rearrange("p (t w) -> p t w", t=T, w=W),
                    in0=iotas.unsqueeze(1).to_broadcast([P, T, W]),
                    in1=lgids[:, grp * T: (grp + 1) * T]
                        .unsqueeze(2).to_broadcast([P, T, W]),
                    op=eq)

            scst = None
            for grp in range(n_groups):
                grp_real = min(NG, n_real - grp * NG)
                Tr = (grp_real + P - 1) // P
                Sr = (grp_real + 511) // 512

                x0c = x0p.tile([P, NG], BF16, tag="x0c")
                nc.sync.dma_start(
                    out=x0c[:, :Sr * 512],
                    in_=x0T[:, grp * NG: grp * NG + Sr * 512])
                el2c = el2p.tile([P, T * ML2], BF16, tag="el2c")
                nc.sync.dma_start(
                    out=el2c[:, :Tr * ML2],
                    in_=embL2[grp][:, :Tr * ML2])
                goff = 0

                if grp % OB == 0:
                    scst = stp.tile([1, OB * NG], F32, tag="scst")
                boff = grp % OB

                # ---- scalar (MLP) branch, 512 nodes at a time ----
                h2list = []
                for s in range(Sr):
                    nsl = slice(goff + s * 512, goff + (s + 1) * 512)
                    h1p = ph1p.tile([P, 512], F32, tag="h1p")
                    nc.tensor.matmul(h1p[:], w1s, x0c[:, nsl],
                                     start=True, stop=True)
                    h1s = hp.tile([P, 512], BF16, tag="h1s")
                    nc.scalar.activation(h1s[:], h1p[:], silu, bias=b1s)
                    h2p = ph2p.tile([P, 512], F32, tag="h2p")
                    nc.tensor.matmul(h2p[:], w2s, h1s[:],
                                     start=True, stop=True)
                    h2s = hp.tile([P, 512], BF16, tag="h2s")
                    nc.scalar.activation(h2s[:], h2p[:], silu, bias=b2s)
                    h2list.append(h2s)

                # ---- l=2 branch with fused segment sum ----
                # cols 0:256 = (m0,m1) -> pS[0:W]; 256:640 = (m2..m4)
                # -> pS[32:32+W]; both accumulate over the group's tiles.
                A8 = Aall[:, grp * T * W: (grp + 1) * T * W]
                pS = pSp.tile([64, 384], F32, tag="pS")
                for t in range(Tr):
                    At = A8[:, t * W:(t + 1) * W]
                    base = t * ML2
                    nc.tensor.matmul(pS[0:W, 0:256], At,
                                     el2c[:, base: base + 256],
                                     start=(t == 0), stop=(t == Tr - 1),
                                     tile_position=(0, 0))
                    nc.tensor.matmul(pS[32:32 + W, 0:384], At,
                                     el2c[:, base + 256: base + ML2],
                                     start=(t == 0), stop=(t == Tr - 1),
                                     tile_position=(0, 32))
                    if t == 0:
                        scp = pscp.tile([P, 512], F32, tag="scp")
                        for s in range(Sr):
                            q = 64 + 32 * s
                            nc.tensor.matmul(scp[q:q + 1, :], w3s,
                                             h2list[s][:],
                                             start=True, stop=True,
                                             tile_position=(0, q))

                # collapse c on the vector engine -> (W, 5) partial
                nc.vector.tensor_reduce(
                    out=anisoSt[0:W, grp * 5: grp * 5 + 2],
                    in_=pS[0:W, 0:256].rearrange("p (f c) -> p f c",
                                                 f=2, c=P),
                    axis=mybir.AxisListType.X,
                    op=mybir.AluOpType.add)
                nc.vector.tensor_reduce(
                    out=anisoSt[32:32 + W, grp * 5 + 2: grp * 5 + 5],
                    in_=pS[32:32 + W, 0:384].rearrange("p (f c) -> p f c",
                                                       f=3, c=P),
                    axis=mybir.AxisListType.X,
                    op=mybir.AluOpType.add)

                last_ob = grp >= (n_groups // OB) * OB
                for s in range(Sr):
                    q = 64 + 32 * s
                    dst = scst[:, boff * NG + s * 512:
                               boff * NG + (s + 1) * 512]
                    if last_ob:
                        nc.scalar.copy(out=dst, in_=scp[q:q + 1, :])
                    else:
                        nc.vector.tensor_copy(out=dst, in_=scp[q:q + 1, :])

                if grp % OB == OB - 1 or grp == n_groups - 1:
                    ob = grp // OB
                    nc.scalar.dma_start(out=scal[ob: ob + 1, :],
                                        in_=scst[:])

            nc.sync.dma_start(out=aniso_o, in_=anisoSt[:])

    nc.compile()
    _BUILD_CACHE[key] = nc
    return nc


def _next_pow2(x):
    p = 8
    while p < x:
        p *= 2
    return p


def _host_reference(node_embedding, W1, b1, W2, b2, W3, b3, w_l2, batch,
                    natoms):
    """Pure-numpy fallback (only used for pathological graph layouts)."""
    G = natoms.shape[0]
    inv = 1.0 / natoms.astype(np.float32)
    x = node_embedding[:, 0, :]
    h = x @ W1.T + b1
    h = h / (1.0 + np.exp(-h))
    h = h @ W2.T + b2
    h = h / (1.0 + np.exp(-h))
    ns = (h @ W3.T + b3)[:, 0]
    ok = (batch >= 0) & (batch < G)
    bok = batch[ok]
    iso = np.bincount(bok, weights=ns[ok], minlength=G).astype(np.float32) \
        * inv
    nl2 = np.einsum("nmc,c->nm", node_embedding[:, 4:9, :], w_l2[0])
    aniso = np.stack(
        [np.bincount(bok, weights=nl2[ok, m], minlength=G)
         for m in range(5)], axis=1).astype(np.float32) * inv[:, None]
    dec = np.concatenate([iso[:, None], np.zeros((G, 3), np.float32), aniso],
                         axis=1)
    return (dec @ _CG).reshape(-1, 3, 3).astype(np.float32)


def kernel(node_embedding, W1, b1, W2, b2, W3, b3, w_l2, batch, natoms):
    node_embedding = np.asarray(node_embedding, dtype=np.float32)
    W1 = np.asarray(W1, dtype=np.float32)
    b1 = np.asarray(b1, dtype=np.float32)
    W2 = np.asarray(W2, dtype=np.float32)
    b2 = np.asarray(b2, dtype=np.float32)
    W3 = np.asarray(W3, dtype=np.float32)
    b3 = np.asarray(b3, dtype=np.float32)
    w_l2 = np.asarray(w_l2, dtype=np.float32)
    batch = np.asarray(batch).astype(np.int64)
    natoms_in = np.asarray(natoms)

    N = node_embedding.shape[0]
    G = natoms_in.shape[0]
    n_sh = (N + N_CORES - 1) // N_CORES
    n_groups = (n_sh + NG - 1) // NG
    n_pad = n_groups * NG
    n_sc = (n_groups + SC - 1) // SC

    # per-core shard ranges and group graph bases
    shards = []
    W_need = 8
    for c in range(N_CORES):
        n0 = min(c * n_sh, N)
        n1 = min(n0 + n_sh, N)
        b = batch[n0:n1]
        nreal = n1 - n0
        gbase = np.zeros(n_groups, np.int64)
        for grp in range(n_groups):
            lo = grp * NG
            hi = min(lo + NG, nreal)
            if lo < nreal:
                gbase[grp] = b[lo]
                span = int(b[hi - 1] - b[lo] + 1)
                W_need = max(W_need, span)
        shards.append((n0, n1, b, gbase))
    W = _next_pow2(W_need)
    if (W > 32 or not np.all(batch[:-1] <= batch[1:])
            or batch.min(initial=0) < 0 or batch.max(initial=0) >= G):
        return _host_reference(node_embedding, W1, b1, W2, b2, W3, b3,
                               w_l2, batch, natoms_in)

    nc = _build(n_pad, n_groups, W, n_sh)

    w1t = W1.T.astype(WIRE)
    w2t = W2.T.astype(WIRE)
    w3t = W3.T.astype(WIRE)
    cf32 = np.ascontiguousarray(np.stack([b1, b2], axis=1))
    iota_c = np.tile(np.arange(W, dtype=np.float32), (P, 1)).astype(WIRE)

    in_maps = []
    for c in range(N_CORES):
        n0, n1, b, gbase = shards[c]
        nreal = n1 - n0
        x0T = np.zeros((P, n_pad), WIRE)
        x0T[:, :nreal] = node_embedding[n0:n1, 0, :].T.astype(WIRE)
        # pre-tiled l=2 data scaled by w_l2: [grp, p, (t, m)]
        el2 = np.zeros((n_pad, ML2), WIRE)
        el2[:nreal] = (node_embedding[n0:n1, 4:9, :]
                       * w_l2[0]).reshape(nreal, ML2).astype(WIRE)
        el2 = np.ascontiguousarray(
            el2.reshape(n_groups, T, P, ML2).transpose(0, 2, 1, 3)
               .reshape(n_groups, P, T * ML2))
        lg = np.full(n_pad, -1.0, np.float32)
        lg[:nreal] = (b - np.repeat(gbase, NG)[:nreal]).astype(np.float32)
        lg_t = lg.reshape(n_groups, T, P).transpose(2, 0, 1) \
            .reshape(P, n_groups * T).astype(WIRE)
        cbf = np.ascontiguousarray(
            np.concatenate([w1t, w2t, w3t, iota_c, lg_t], axis=1))
        in_maps.append({
            "x0T": x0T, "embL2": el2, "cbf": cbf, "cf32": cf32,
        })

    res = bass_utils.run_bass_kernel_spmd(nc, in_maps,
                                          core_ids=list(range(N_CORES)))

    # ---- host epilogue ----
    inv = (1.0 / natoms_in.astype(np.float32)).astype(np.float32)
    node_scalar = np.empty(N, np.float32)
    aniso = np.zeros((G + 64, 5), np.float32)
    for c in range(N_CORES):
        n0, n1, _, gbase = shards[c]
        nreal = n1 - n0
        sc = res.results[c]["scal"].reshape(-1)[:nreal]
        node_scalar[n0:n1] = sc
        an = res.results[c]["aniso_o"]       # (48, n_groups*5)
        for grp in range(n_groups):
            if grp * NG < nreal:
                gb = int(gbase[grp])
                aniso[gb:gb + W, 0:2] += an[0:W, grp * 5: grp * 5 + 2]
                aniso[gb:gb + W, 2:5] += an[32:32 + W,
                                            grp * 5 + 2: grp * 5 + 5]
    iso = np.bincount(batch, weights=node_scalar + b3[0], minlength=G)
    iso = iso.astype(np.float32) * inv
    aniso = aniso[:G] * inv[:, None]
    dec = np.concatenate([iso[:, None], np.zeros((G, 3), np.float32), aniso],
                         axis=1)
    return (dec @ _CG).reshape(-1, 3, 3).astype(np.float32)
